# revision 3
# baseline (speedup 1.0000x reference)
"""KAN layer (spline order k=3, grid G=5, uniform knots) on 8 TRN2 NeuronCores.

Math: the reference's per-unit B-spline basis uses the SAME uniform knot
vector (step h=0.4 on [-2.2, 2.2]) for every (out,in) unit, so the 8 cubic
basis functions are translates of the cardinal cubic B-spline N3:

    B_c(t) = N3(s - c),  s = (t + 2.2)/0.4 = 2.5 t + 5.5
    N3(u)  = z^3/6 - (2/3) relu(z-1)^3,   z = relu(2 - |u - 2|)

(exact, cancellation-free, and reproduces the half-open-interval zero
outside the knot span).  The whole layer is then

    out[b,o] = bias[o] + sum_i W[o,i] silu(x[b,i])
             + sum_{i,c} C6[o,i,c] * phi[b,i,c]

with W = (mask*scale_base), C6 = (mask*scale_spline/6)*coeff and
phi = 6*N3 = z^3 - 4*relu(z-1)^3.  Both contractions are matmuls over
i (K=128) done on the tensor engine with PSUM accumulation.

Sharding: pure data-parallel over batch (512 -> 64 per core); weights are
replicated, no collectives.  Host work is layout-only (transpose/reshape);
all arithmetic (including mask/scale folding) happens on-device.
"""

from contextlib import ExitStack

import numpy as np

import concourse.bacc as bacc
import concourse.bass as bass
import concourse.mybir as mybir
import concourse.tile as tile
from concourse.bass_utils import run_bass_kernel_spmd

B, IN, OUT, NCORE = 512, 128, 128, 8
BL = B // NCORE  # 64 batch rows per core
GK = 8           # G + K basis functions per unit
F32 = mybir.dt.float32
AF = mybir.ActivationFunctionType
OP = mybir.AluOpType

_CACHE = {}


def _build_nc():
    nc = bacc.Bacc(
        "TRN2",
        target_bir_lowering=False,
        debug=False,
        enable_asserts=False,
        num_devices=NCORE,
    )
    xt_d = nc.dram_tensor("xt", [IN, BL], F32, kind="ExternalInput").ap()
    ct_d = nc.dram_tensor("coefft", [IN, GK, OUT], F32, kind="ExternalInput").ap()
    mk_d = nc.dram_tensor("maskt", [IN, OUT], F32, kind="ExternalInput").ap()
    sb_d = nc.dram_tensor("sbaset", [IN, OUT], F32, kind="ExternalInput").ap()
    ss_d = nc.dram_tensor("ssplinet", [IN, OUT], F32, kind="ExternalInput").ap()
    bi_d = nc.dram_tensor("biasp", [OUT, 1], F32, kind="ExternalInput").ap()
    out_d = nc.dram_tensor("outt", [OUT, BL], F32, kind="ExternalOutput").ap()

    with tile.TileContext(nc) as tc, ExitStack() as ctx:
        pool = ctx.enter_context(tc.tile_pool(name="main", bufs=1))
        psum = ctx.enter_context(
            tc.tile_pool(name="psum", bufs=1, space=bass.MemorySpace.PSUM)
        )

        # ---- loads ----
        xt = pool.tile([IN, BL], F32)
        nc.sync.dma_start(xt[:], xt_d)
        cw = pool.tile([IN, GK, OUT], F32)
        nc.sync.dma_start(cw[:], ct_d)
        mk = pool.tile([IN, OUT], F32)
        nc.sync.dma_start(mk[:], mk_d)
        sbt = pool.tile([IN, OUT], F32)
        nc.sync.dma_start(sbt[:], sb_d)
        sst = pool.tile([IN, OUT], F32)
        nc.sync.dma_start(sst[:], ss_d)
        bi = pool.tile([OUT, 1], F32)
        nc.sync.dma_start(bi[:], bi_d)

        # const per-partition bias columns for ACT ops
        cb0 = pool.tile([128, 1], F32)
        nc.gpsimd.memset(cb0[:], 0.0)
        cbm2 = pool.tile([128, 1], F32)
        nc.gpsimd.memset(cbm2[:], -2.0)
        cb2 = pool.tile([128, 1], F32)
        nc.gpsimd.memset(cb2[:], 2.0)
        cbm1 = pool.tile([128, 1], F32)
        nc.gpsimd.memset(cbm1[:], -1.0)

        # ---- silu(x) = x * sigmoid(x) (Sigmoid shares an act table with
        # Abs/Relu/Square/Identity, so no table switches) ----
        sg = pool.tile([IN, BL], F32)
        nc.scalar.activation(sg[:], xt[:], AF.Sigmoid, bias=cb0[:], scale=1.0)
        sx = pool.tile([IN, BL], F32)
        nc.vector.tensor_mul(sx[:], xt[:], sg[:])

        # ---- basis: phi[:, c, :] = z^3 - 4*relu(z-1)^3, z = relu(2-|s-c-2|) ----
        u = pool.tile([IN, GK, BL], F32)
        for c in range(GK):
            # u_c = 2.5*x + (5.5 - c)
            nc.vector.tensor_scalar(
                u[:, c, :], xt[:], 2.5, 5.5 - float(c), OP.mult, OP.add
            )
        v = pool.tile([IN, GK, BL], F32)
        nc.scalar.activation(v[:], u[:], AF.Abs, bias=cbm2[:], scale=1.0)
        z = pool.tile([IN, GK, BL], F32)
        nc.scalar.activation(z[:], v[:], AF.Relu, bias=cb2[:], scale=-1.0)
        y = pool.tile([IN, GK, BL], F32)
        nc.scalar.activation(y[:], z[:], AF.Relu, bias=cbm1[:], scale=1.0)
        z2 = pool.tile([IN, GK, BL], F32)
        nc.scalar.activation(z2[:], z[:], AF.Square, bias=cb0[:], scale=1.0)
        y2 = pool.tile([IN, GK, BL], F32)
        nc.scalar.activation(y2[:], y[:], AF.Square, bias=cb0[:], scale=1.0)
        z3 = pool.tile([IN, GK, BL], F32)
        nc.vector.tensor_mul(z3[:], z2[:], z[:])
        y3 = pool.tile([IN, GK, BL], F32)
        nc.vector.tensor_mul(y3[:], y2[:], y[:])
        phi = pool.tile([IN, GK, BL], F32)
        # phi = (y3 * -4) + z3
        nc.vector.scalar_tensor_tensor(phi[:], y3[:], -4.0, z3[:], OP.mult, OP.add)

        # ---- fold mask/scales into weights (on gpsimd; DVE is busy) ----
        msb = pool.tile([IN, OUT], F32)
        nc.gpsimd.tensor_mul(msb[:], mk[:], sbt[:])
        mss6 = pool.tile([IN, OUT], F32)
        # (ss * 1/6) * mask  (scalar_tensor_tensor not supported on Pool)
        nc.vector.scalar_tensor_tensor(
            mss6[:], sst[:], 1.0 / 6.0, mk[:], OP.mult, OP.mult
        )
        cwm = pool.tile([IN, GK, OUT], F32)
        for c in range(GK):
            nc.gpsimd.tensor_mul(cwm[:, c, :], cw[:, c, :], mss6[:])

        # ---- matmuls: accumulate silu term + 8 spline terms into PSUM ----
        ps = psum.tile([OUT, BL], F32)
        nc.tensor.matmul(ps[:], msb[:], sx[:], start=True, stop=False)
        for c in range(GK):
            nc.tensor.matmul(
                ps[:], cwm[:, c, :], phi[:, c, :], start=False, stop=(c == GK - 1)
            )

        # ---- + bias, PSUM -> SBUF -> DRAM ----
        ob = pool.tile([OUT, BL], F32)
        nc.scalar.activation(ob[:], ps[:], AF.Identity, bias=bi[:], scale=1.0)
        nc.sync.dma_start(out_d, ob[:])

    nc.compile()
    return nc


def _prep_in_maps(x, coeff, mask, scale_base, scale_spline, bias):
    f32 = np.float32
    x = np.asarray(x, f32)
    coeff = np.asarray(coeff, f32)
    mask = np.asarray(mask, f32)
    scale_base = np.asarray(scale_base, f32)
    scale_spline = np.asarray(scale_spline, f32)
    bias = np.asarray(bias, f32)

    xT = np.ascontiguousarray(x.T)  # [IN, B]
    # coeff[s, c], s = o*IN + i  ->  [i, c, o]
    coefft = np.ascontiguousarray(coeff.reshape(OUT, IN, GK).transpose(1, 2, 0))
    maskt = np.ascontiguousarray(mask.reshape(OUT, IN).T)
    sbaset = np.ascontiguousarray(scale_base.reshape(OUT, IN).T)
    ssplinet = np.ascontiguousarray(scale_spline.reshape(OUT, IN).T)
    biasp = np.ascontiguousarray(bias.reshape(OUT, 1))

    in_maps = []
    for j in range(NCORE):
        in_maps.append(
            {
                "xt": np.ascontiguousarray(xT[:, j * BL : (j + 1) * BL]),
                "coefft": coefft,
                "maskt": maskt,
                "sbaset": sbaset,
                "ssplinet": ssplinet,
                "biasp": biasp,
            }
        )
    return in_maps


def run(x, coeff, mask, scale_base, scale_spline, bias, trace=False):
    if "nc" not in _CACHE:
        _CACHE["nc"] = _build_nc()
    nc = _CACHE["nc"]
    in_maps = _prep_in_maps(x, coeff, mask, scale_base, scale_spline, bias)
    res = run_bass_kernel_spmd(
        nc, in_maps, core_ids=list(range(NCORE)), trace=trace
    )
    outT = np.concatenate(
        [res.results[j]["outt"] for j in range(NCORE)], axis=1
    )  # [OUT, B]
    return np.ascontiguousarray(outT.T), res


def kernel(x, grid, coeff, mask, scale_base, scale_spline, bias, k):
    assert int(np.asarray(k)) == 3
    out, _ = run(x, coeff, mask, scale_base, scale_spline, bias, trace=False)
    return out


# revision 7
# speedup vs baseline: 1.0301x; 1.0301x over previous
"""KAN layer (spline order k=3, grid G=5, uniform knots) on 8 TRN2 NeuronCores.

Math: the reference's per-unit B-spline basis uses the SAME uniform knot
vector (step h=0.4 on [-2.2, 2.2]) for every (out,in) unit, so the 8 cubic
basis functions are translates of the cardinal cubic B-spline N3:

    B_c(t) = N3(s - c),  s = (t + 2.2)/0.4 = 2.5 t + 5.5
    N3(u)  = z^3/6 - (2/3) relu(z-1)^3,   z = relu(2 - |u - 2|)

(exact, cancellation-free, and reproduces the half-open-interval zero
outside the knot span).  With v = |u-2|: z = relu(2-v), y = relu(1-v)
(y == relu(z-1), but computable straight from v).  The layer is then

    out[b,o] = bias[o] + sum_i W[o,i] silu(x[b,i])
             + sum_{i,c} C6[o,i,c] * phi[b,i,c]

with W = (mask*scale_base), C6 = (mask*scale_spline/6)*coeff and
phi = 6*N3 = z^3 - 4*y^3.  Both contractions are matmuls over i (K=128)
on the tensor engine with PSUM accumulation.

Sharding: pure data-parallel over batch (512 -> 64 per core); weights are
replicated, no collectives.  Host work is layout-only (transpose/reshape);
all arithmetic (including mask/scale folding) happens on-device.
"""

from contextlib import ExitStack

import numpy as np

import concourse.bacc as bacc
import concourse.bass as bass
import concourse.mybir as mybir
import concourse.tile as tile
from concourse.bass_utils import run_bass_kernel_spmd

B, IN, OUT, NCORE = 512, 128, 128, 8
BL = B // NCORE  # 64 batch rows per core
GK = 8           # G + K basis functions per unit
HC = GK // 2     # half of the c-range, for pipelining
F32 = mybir.dt.float32
F32R = mybir.dt.float32r
AF = mybir.ActivationFunctionType
OP = mybir.AluOpType

USE_POW = False   # DVE pow for cubes fails walrus ISA check on TRN2
USE_F32R = True   # single-pass reduced-precision f32 matmul

_CACHE = {}


def _build_nc():
    nc = bacc.Bacc(
        "TRN2",
        target_bir_lowering=False,
        debug=False,
        enable_asserts=False,
        num_devices=NCORE,
    )
    xt_d = nc.dram_tensor("xt", [IN, BL], F32, kind="ExternalInput").ap()
    ct_d = nc.dram_tensor("coefft", [IN, GK, OUT], F32, kind="ExternalInput").ap()
    # mask / scale_base / scale_spline concatenated: [IN, 3, OUT]
    sc_d = nc.dram_tensor("scales3", [IN, 3, OUT], F32, kind="ExternalInput").ap()
    bi_d = nc.dram_tensor("biasp", [OUT, 1], F32, kind="ExternalInput").ap()
    out_d = nc.dram_tensor("outt", [OUT, BL], F32, kind="ExternalOutput").ap()

    MMT = F32R if USE_F32R else F32  # matmul-operand tiles: walrus requires
    # fp32r-matmul inputs to be *written* as fp32r by their producers

    with tile.TileContext(nc) as tc, ExitStack() as ctx:
        pool = ctx.enter_context(tc.tile_pool(name="main", bufs=1))
        psum = ctx.enter_context(
            tc.tile_pool(name="psum", bufs=1, space=bass.MemorySpace.PSUM)
        )

        # ---- loads, spread across engines so issue costs overlap ----
        xt = pool.tile([IN, BL], F32)
        nc.sync.dma_start(xt[:], xt_d)
        sc = pool.tile([IN, 3, OUT], F32)
        nc.gpsimd.dma_start(sc[:], sc_d)
        cw = pool.tile([IN, GK, OUT], F32)
        nc.gpsimd.dma_start(cw[:], ct_d)
        bi = pool.tile([OUT, 1], F32)
        nc.scalar.dma_start(bi[:], bi_d)
        mk, sbt, sst = sc[:, 0, :], sc[:, 1, :], sc[:, 2, :]

        # const per-partition bias columns for ACT ops
        cb0 = pool.tile([128, 1], F32)
        nc.gpsimd.memset(cb0[:], 0.0)
        cbm2 = pool.tile([128, 1], F32)
        nc.gpsimd.memset(cbm2[:], -2.0)
        cb2 = pool.tile([128, 1], F32)
        nc.gpsimd.memset(cb2[:], 2.0)
        cb1 = pool.tile([128, 1], F32)
        nc.gpsimd.memset(cb1[:], 1.0)

        # ---- silu(x) = x * sigmoid(x) ----
        sg = pool.tile([IN, BL], F32)
        nc.scalar.activation(sg[:], xt[:], AF.Sigmoid, bias=cb0[:], scale=1.0)
        sx = pool.tile([IN, BL], MMT)
        nc.vector.tensor_mul(sx[:], xt[:], sg[:])

        # ---- basis, in two c-halves for pipelining ----
        u = pool.tile([IN, GK, BL], F32)
        for c in range(GK):
            nc.vector.tensor_scalar(
                u[:, c, :], xt[:], 2.5, 5.5 - float(c), OP.mult, OP.add
            )
        v = pool.tile([IN, GK, BL], F32)
        z = pool.tile([IN, GK, BL], F32)
        y = pool.tile([IN, GK, BL], F32)
        z3 = pool.tile([IN, GK, BL], F32)
        y3 = pool.tile([IN, GK, BL], F32)
        phi = pool.tile([IN, GK, BL], MMT)
        if not USE_POW:
            z2 = pool.tile([IN, GK, BL], F32)
            y2 = pool.tile([IN, GK, BL], F32)
        for h in range(2):
            hs = slice(h * HC, (h + 1) * HC)
            nc.scalar.activation(v[:, hs, :], u[:, hs, :], AF.Abs, bias=cbm2[:], scale=1.0)
            nc.scalar.activation(z[:, hs, :], v[:, hs, :], AF.Relu, bias=cb2[:], scale=-1.0)
            nc.scalar.activation(y[:, hs, :], v[:, hs, :], AF.Relu, bias=cb1[:], scale=-1.0)
            if USE_POW:
                nc.vector.tensor_scalar(z3[:, hs, :], z[:, hs, :], 3.0, None, OP.pow)
                nc.vector.tensor_scalar(y3[:, hs, :], y[:, hs, :], 3.0, None, OP.pow)
            else:
                nc.scalar.activation(z2[:, hs, :], z[:, hs, :], AF.Square, bias=cb0[:], scale=1.0)
                nc.scalar.activation(y2[:, hs, :], y[:, hs, :], AF.Square, bias=cb0[:], scale=1.0)
                nc.vector.tensor_mul(z3[:, hs, :], z2[:, hs, :], z[:, hs, :])
                nc.vector.tensor_mul(y3[:, hs, :], y2[:, hs, :], y[:, hs, :])
            # phi = (y3 * -4) + z3
            nc.vector.scalar_tensor_tensor(
                phi[:, hs, :], y3[:, hs, :], -4.0, z3[:, hs, :], OP.mult, OP.add
            )

        # ---- fold mask/scales into weights ----
        mss6 = pool.tile([IN, OUT], F32)
        nc.vector.scalar_tensor_tensor(
            mss6[:], sst[:], 1.0 / 6.0, mk[:], OP.mult, OP.mult
        )
        msb = pool.tile([IN, OUT], MMT)
        nc.gpsimd.tensor_mul(msb[:], mk[:], sbt[:])
        cwm = pool.tile([IN, GK, OUT], MMT)
        for c in range(GK):
            nc.gpsimd.tensor_mul(cwm[:, c, :], cw[:, c, :], mss6[:])

        # ---- matmuls: silu term + 8 spline terms accumulate in PSUM ----
        ps = psum.tile([OUT, BL], F32)
        nc.tensor.matmul(ps[:], msb[:], sx[:], start=True, stop=False)
        for c in range(GK):
            nc.tensor.matmul(
                ps[:],
                cwm[:, c, :],
                phi[:, c, :],
                start=False,
                stop=(c == GK - 1),
            )

        # ---- + bias, PSUM -> SBUF -> DRAM ----
        ob = pool.tile([OUT, BL], F32)
        nc.scalar.activation(ob[:], ps[:], AF.Identity, bias=bi[:], scale=1.0)
        nc.sync.dma_start(out_d, ob[:])

    nc.compile()
    return nc


def _prep_in_maps(x, coeff, mask, scale_base, scale_spline, bias):
    f32 = np.float32
    x = np.asarray(x, f32)
    coeff = np.asarray(coeff, f32)
    mask = np.asarray(mask, f32)
    scale_base = np.asarray(scale_base, f32)
    scale_spline = np.asarray(scale_spline, f32)
    bias = np.asarray(bias, f32)

    xT = np.ascontiguousarray(x.T)  # [IN, B]
    # coeff[s, c], s = o*IN + i  ->  [i, c, o]
    coefft = np.ascontiguousarray(coeff.reshape(OUT, IN, GK).transpose(1, 2, 0))
    scales3 = np.ascontiguousarray(
        np.stack(
            [
                mask.reshape(OUT, IN).T,
                scale_base.reshape(OUT, IN).T,
                scale_spline.reshape(OUT, IN).T,
            ],
            axis=1,
        )
    )  # [IN, 3, OUT]
    biasp = np.ascontiguousarray(bias.reshape(OUT, 1))

    in_maps = []
    for j in range(NCORE):
        in_maps.append(
            {
                "xt": np.ascontiguousarray(xT[:, j * BL : (j + 1) * BL]),
                "coefft": coefft,
                "scales3": scales3,
                "biasp": biasp,
            }
        )
    return in_maps


def run(x, coeff, mask, scale_base, scale_spline, bias, trace=False):
    if "nc" not in _CACHE:
        _CACHE["nc"] = _build_nc()
    nc = _CACHE["nc"]
    in_maps = _prep_in_maps(x, coeff, mask, scale_base, scale_spline, bias)
    res = run_bass_kernel_spmd(
        nc, in_maps, core_ids=list(range(NCORE)), trace=trace
    )
    outT = np.concatenate(
        [res.results[j]["outt"] for j in range(NCORE)], axis=1
    )  # [OUT, B]
    return np.ascontiguousarray(outT.T), res


def kernel(x, grid, coeff, mask, scale_base, scale_spline, bias, k):
    assert int(np.asarray(k)) == 3
    out, _ = run(x, coeff, mask, scale_base, scale_spline, bias, trace=False)
    return out


# revision 8
# speedup vs baseline: 1.0699x; 1.0386x over previous
"""KAN layer (spline order k=3, grid G=5, uniform knots) on 8 TRN2 NeuronCores.

Math: the reference's per-unit B-spline basis uses the SAME uniform knot
vector (step h=0.4 on [-2.2, 2.2]) for every (out,in) unit, so the 8 cubic
basis functions are translates of the cardinal cubic B-spline N3:

    B_c(t) = N3(s - c),  s = (t + 2.2)/0.4 = 2.5 t + 5.5
    N3(u)  = z^3/6 - (2/3) relu(z-1)^3,   z = relu(2 - |u - 2|)

(exact, cancellation-free, and reproduces the half-open-interval zero
outside the knot span).  With v = |u-2|: z = relu(2-v), y = relu(1-v)
(y == relu(z-1), but computable straight from v).  The layer is then

    out[b,o] = bias[o] + sum_i W[o,i] silu(x[b,i])
             + sum_{i,c} C6[o,i,c] * phi[b,i,c]

with W = (mask*scale_base), C6 = (mask*scale_spline/6)*coeff and
phi = 6*N3 = z^3 - 4*y^3.  Both contractions are matmuls over i (K=128)
on the tensor engine with PSUM accumulation.

Sharding: pure data-parallel over batch (512 -> 64 per core); weights are
replicated, no collectives.  Host work is layout-only (transpose/reshape);
all arithmetic (including mask/scale folding) happens on-device.
"""

from contextlib import ExitStack

import numpy as np

import concourse.bacc as bacc
import concourse.bass as bass
import concourse.mybir as mybir
import concourse.tile as tile
from concourse.bass_utils import run_bass_kernel_spmd

B, IN, OUT, NCORE = 512, 128, 128, 8
BL = B // NCORE  # 64 batch rows per core
GK = 8           # G + K basis functions per unit
HC = GK // 2     # half of the c-range, for pipelining
F32 = mybir.dt.float32
F32R = mybir.dt.float32r
AF = mybir.ActivationFunctionType
OP = mybir.AluOpType

USE_POW = False   # DVE pow for cubes fails walrus ISA check on TRN2
USE_F32R = False  # fp32r halves PE time but costs 500x accuracy; keep f32

_CACHE = {}


def _build_nc():
    nc = bacc.Bacc(
        "TRN2",
        target_bir_lowering=False,
        debug=False,
        enable_asserts=False,
        num_devices=NCORE,
    )
    xt_d = nc.dram_tensor("xt", [IN, BL], F32, kind="ExternalInput").ap()
    ct_d = nc.dram_tensor("coefft", [IN, GK, OUT], F32, kind="ExternalInput").ap()
    # mask / scale_base / scale_spline concatenated: [IN, 3, OUT]
    sc_d = nc.dram_tensor("scales3", [IN, 3, OUT], F32, kind="ExternalInput").ap()
    bi_d = nc.dram_tensor("biasp", [OUT, 1], F32, kind="ExternalInput").ap()
    out_d = nc.dram_tensor("outt", [OUT, BL], F32, kind="ExternalOutput").ap()

    MMT = F32R if USE_F32R else F32  # matmul-operand tiles: walrus requires
    # fp32r-matmul inputs to be *written* as fp32r by their producers

    with tile.TileContext(nc) as tc, ExitStack() as ctx:
        pool = ctx.enter_context(tc.tile_pool(name="main", bufs=1))
        psum = ctx.enter_context(
            tc.tile_pool(name="psum", bufs=1, space=bass.MemorySpace.PSUM)
        )

        # ---- loads, spread across engines so issue costs overlap ----
        xt = pool.tile([IN, BL], F32)
        nc.sync.dma_start(xt[:], xt_d)
        sc = pool.tile([IN, 3, OUT], F32)
        nc.gpsimd.dma_start(sc[:], sc_d)
        cw = pool.tile([IN, GK, OUT], F32)
        nc.gpsimd.dma_start(cw[:], ct_d)
        bi = pool.tile([OUT, 1], F32)
        nc.scalar.dma_start(bi[:], bi_d)
        mk, sbt, sst = sc[:, 0, :], sc[:, 1, :], sc[:, 2, :]

        # const per-partition bias columns for ACT ops
        cb0 = pool.tile([128, 1], F32)
        nc.gpsimd.memset(cb0[:], 0.0)
        cbm2 = pool.tile([128, 1], F32)
        nc.gpsimd.memset(cbm2[:], -2.0)
        cb2 = pool.tile([128, 1], F32)
        nc.gpsimd.memset(cb2[:], 2.0)
        cb1 = pool.tile([128, 1], F32)
        nc.gpsimd.memset(cb1[:], 1.0)

        # ---- silu(x) = x * sigmoid(x) ----
        sg = pool.tile([IN, BL], F32)
        nc.scalar.activation(sg[:], xt[:], AF.Sigmoid, bias=cb0[:], scale=1.0)
        sx = pool.tile([IN, BL], MMT)
        nc.vector.tensor_mul(sx[:], xt[:], sg[:])

        # ---- basis, in two c-halves for pipelining ----
        u = pool.tile([IN, GK, BL], F32)
        for c in range(GK):
            nc.vector.tensor_scalar(
                u[:, c, :], xt[:], 2.5, 5.5 - float(c), OP.mult, OP.add
            )
        v = pool.tile([IN, GK, BL], F32)
        z = pool.tile([IN, GK, BL], F32)
        y = pool.tile([IN, GK, BL], F32)
        z3 = pool.tile([IN, GK, BL], F32)
        y3 = pool.tile([IN, GK, BL], F32)
        phi = pool.tile([IN, GK, BL], MMT)
        if not USE_POW:
            z2 = pool.tile([IN, GK, BL], F32)
            y2 = pool.tile([IN, GK, BL], F32)
        for h in range(2):
            hs = slice(h * HC, (h + 1) * HC)
            nc.scalar.activation(v[:, hs, :], u[:, hs, :], AF.Abs, bias=cbm2[:], scale=1.0)
            nc.scalar.activation(z[:, hs, :], v[:, hs, :], AF.Relu, bias=cb2[:], scale=-1.0)
            nc.scalar.activation(y[:, hs, :], v[:, hs, :], AF.Relu, bias=cb1[:], scale=-1.0)
            if USE_POW:
                nc.vector.tensor_scalar(z3[:, hs, :], z[:, hs, :], 3.0, None, OP.pow)
                nc.vector.tensor_scalar(y3[:, hs, :], y[:, hs, :], 3.0, None, OP.pow)
            else:
                nc.scalar.activation(z2[:, hs, :], z[:, hs, :], AF.Square, bias=cb0[:], scale=1.0)
                nc.scalar.activation(y2[:, hs, :], y[:, hs, :], AF.Square, bias=cb0[:], scale=1.0)
                nc.vector.tensor_mul(z3[:, hs, :], z2[:, hs, :], z[:, hs, :])
                nc.vector.tensor_mul(y3[:, hs, :], y2[:, hs, :], y[:, hs, :])
            # phi = (y3 * -4) + z3
            nc.vector.scalar_tensor_tensor(
                phi[:, hs, :], y3[:, hs, :], -4.0, z3[:, hs, :], OP.mult, OP.add
            )

        # ---- fold mask/scales into weights ----
        mss6 = pool.tile([IN, OUT], F32)
        nc.vector.scalar_tensor_tensor(
            mss6[:], sst[:], 1.0 / 6.0, mk[:], OP.mult, OP.mult
        )
        msb = pool.tile([IN, OUT], MMT)
        nc.gpsimd.tensor_mul(msb[:], mk[:], sbt[:])
        cwm = pool.tile([IN, GK, OUT], MMT)
        for c in range(GK):
            nc.gpsimd.tensor_mul(cwm[:, c, :], cw[:, c, :], mss6[:])

        # ---- matmuls: silu term + 8 spline terms accumulate in PSUM ----
        ps = psum.tile([OUT, BL], F32)
        nc.tensor.matmul(ps[:], msb[:], sx[:], start=True, stop=False)
        for c in range(GK):
            nc.tensor.matmul(
                ps[:],
                cwm[:, c, :],
                phi[:, c, :],
                start=False,
                stop=(c == GK - 1),
            )

        # ---- + bias, PSUM -> SBUF -> DRAM ----
        ob = pool.tile([OUT, BL], F32)
        nc.scalar.activation(ob[:], ps[:], AF.Identity, bias=bi[:], scale=1.0)
        nc.sync.dma_start(out_d, ob[:])

    nc.compile()
    return nc


def _prep_in_maps(x, coeff, mask, scale_base, scale_spline, bias):
    f32 = np.float32
    x = np.asarray(x, f32)
    coeff = np.asarray(coeff, f32)
    mask = np.asarray(mask, f32)
    scale_base = np.asarray(scale_base, f32)
    scale_spline = np.asarray(scale_spline, f32)
    bias = np.asarray(bias, f32)

    xT = np.ascontiguousarray(x.T)  # [IN, B]
    # coeff[s, c], s = o*IN + i  ->  [i, c, o]
    coefft = np.ascontiguousarray(coeff.reshape(OUT, IN, GK).transpose(1, 2, 0))
    scales3 = np.ascontiguousarray(
        np.stack(
            [
                mask.reshape(OUT, IN).T,
                scale_base.reshape(OUT, IN).T,
                scale_spline.reshape(OUT, IN).T,
            ],
            axis=1,
        )
    )  # [IN, 3, OUT]
    biasp = np.ascontiguousarray(bias.reshape(OUT, 1))

    in_maps = []
    for j in range(NCORE):
        in_maps.append(
            {
                "xt": np.ascontiguousarray(xT[:, j * BL : (j + 1) * BL]),
                "coefft": coefft,
                "scales3": scales3,
                "biasp": biasp,
            }
        )
    return in_maps


def run(x, coeff, mask, scale_base, scale_spline, bias, trace=False):
    if "nc" not in _CACHE:
        _CACHE["nc"] = _build_nc()
    nc = _CACHE["nc"]
    in_maps = _prep_in_maps(x, coeff, mask, scale_base, scale_spline, bias)
    res = run_bass_kernel_spmd(
        nc, in_maps, core_ids=list(range(NCORE)), trace=trace
    )
    outT = np.concatenate(
        [res.results[j]["outt"] for j in range(NCORE)], axis=1
    )  # [OUT, B]
    return np.ascontiguousarray(outT.T), res


def kernel(x, grid, coeff, mask, scale_base, scale_spline, bias, k):
    assert int(np.asarray(k)) == 3
    out, _ = run(x, coeff, mask, scale_base, scale_spline, bias, trace=False)
    return out


# revision 9
# speedup vs baseline: 1.0945x; 1.0230x over previous
"""KAN layer (spline order k=3, grid G=5, uniform knots) on 8 TRN2 NeuronCores.

Math: the reference's per-unit B-spline basis uses the SAME uniform knot
vector (step h=0.4 on [-2.2, 2.2]) for every (out,in) unit, so the 8 cubic
basis functions are translates of the cardinal cubic B-spline N3:

    B_c(t) = N3(s - c),  s = (t + 2.2)/0.4 = 2.5 t + 5.5
    N3(u)  = z^3/6 - (2/3) relu(z-1)^3,   z = relu(2 - |u - 2|)

(exact, cancellation-free, and reproduces the half-open-interval zero
outside the knot span).  With v = |u-2|: z = relu(2-v), y = relu(1-v)
(y == relu(z-1), but computable straight from v).  The layer is then

    out[b,o] = bias[o] + sum_i W[o,i] silu(x[b,i])
             + sum_{i,c} C6[o,i,c] * phi[b,i,c]

with W = (mask*scale_base), C6 = (mask*scale_spline/6)*coeff and
phi = 6*N3 = z^3 - 4*y^3.  Both contractions are matmuls over i (K=128)
on the tensor engine with PSUM accumulation.

Sharding: pure data-parallel over batch (512 -> 64 per core); weights are
replicated, no collectives.  Host work is layout-only (transpose/reshape);
all arithmetic (including mask/scale folding) happens on-device.
"""

from contextlib import ExitStack

import numpy as np

import concourse.bacc as bacc
import concourse.bass as bass
import concourse.mybir as mybir
import concourse.tile as tile
from concourse.bass_utils import run_bass_kernel_spmd

B, IN, OUT, NCORE = 512, 128, 128, 8
BL = B // NCORE  # 64 batch rows per core
GK = 8           # G + K basis functions per unit
HC = GK // 2     # half of the c-range, for pipelining
F32 = mybir.dt.float32
F32R = mybir.dt.float32r
AF = mybir.ActivationFunctionType
OP = mybir.AluOpType

USE_POW = False   # DVE pow for cubes fails walrus ISA check on TRN2
USE_F32R = False  # fp32r halves PE time but costs 500x accuracy; keep f32

_CACHE = {}


def _build_nc():
    nc = bacc.Bacc(
        "TRN2",
        target_bir_lowering=False,
        debug=False,
        enable_asserts=False,
        num_devices=NCORE,
    )
    xt_d = nc.dram_tensor("xt", [IN, BL], F32, kind="ExternalInput").ap()
    ct_d = nc.dram_tensor("coefft", [IN, GK, OUT], F32, kind="ExternalInput").ap()
    # mask / scale_base / scale_spline concatenated: [IN, 3, OUT]
    sc_d = nc.dram_tensor("scales3", [IN, 3, OUT], F32, kind="ExternalInput").ap()
    bi_d = nc.dram_tensor("biasp", [OUT, 1], F32, kind="ExternalInput").ap()
    out_d = nc.dram_tensor("outt", [OUT, BL], F32, kind="ExternalOutput").ap()

    MMT = F32R if USE_F32R else F32  # matmul-operand tiles: walrus requires
    # fp32r-matmul inputs to be *written* as fp32r by their producers

    with tile.TileContext(nc) as tc, ExitStack() as ctx:
        pool = ctx.enter_context(tc.tile_pool(name="main", bufs=1))
        psum = ctx.enter_context(
            tc.tile_pool(name="psum", bufs=1, space=bass.MemorySpace.PSUM)
        )

        # ---- loads, spread across engines so issue costs overlap ----
        xt = pool.tile([IN, BL], F32)
        nc.sync.dma_start(xt[:], xt_d)
        cw = pool.tile([IN, GK, OUT], F32)
        nc.gpsimd.dma_start(cw[:], ct_d)
        sc = pool.tile([IN, 3, OUT], F32)
        nc.gpsimd.dma_start(sc[:], sc_d)
        bi = pool.tile([OUT, 1], F32)
        nc.scalar.dma_start(bi[:], bi_d)
        mk, sbt, sst = sc[:, 0, :], sc[:, 1, :], sc[:, 2, :]

        # const per-partition bias columns for ACT ops
        cb0 = pool.tile([128, 1], F32)
        nc.gpsimd.memset(cb0[:], 0.0)
        cbm2 = pool.tile([128, 1], F32)
        nc.gpsimd.memset(cbm2[:], -2.0)
        cb2 = pool.tile([128, 1], F32)
        nc.gpsimd.memset(cb2[:], 2.0)
        cb1 = pool.tile([128, 1], F32)
        nc.gpsimd.memset(cb1[:], 1.0)

        # ---- 2*silu(x) = x + x*tanh(x/2); the 0.5 is folded into msb.
        # Tanh shares one act table with Abs/Relu/Square/Identity, so the
        # scalar engine loads a single table (sigmoid needed a second). ----
        th = pool.tile([IN, BL], F32)
        nc.scalar.activation(th[:], xt[:], AF.Tanh, bias=cb0[:], scale=0.5)
        sx = pool.tile([IN, BL], MMT)
        nc.vector.scalar_tensor_tensor(sx[:], th[:], 1.0, xt[:], OP.add, OP.mult)

        # ---- basis, in two c-halves for pipelining ----
        u = pool.tile([IN, GK, BL], F32)
        for c in range(GK):
            nc.vector.tensor_scalar(
                u[:, c, :], xt[:], 2.5, 5.5 - float(c), OP.mult, OP.add
            )
        v = pool.tile([IN, GK, BL], F32)
        z = pool.tile([IN, GK, BL], F32)
        y = pool.tile([IN, GK, BL], F32)
        z3 = pool.tile([IN, GK, BL], F32)
        y3 = pool.tile([IN, GK, BL], F32)
        phi = pool.tile([IN, GK, BL], MMT)
        if not USE_POW:
            z2 = pool.tile([IN, GK, BL], F32)
            y2 = pool.tile([IN, GK, BL], F32)
        for h in range(2):
            hs = slice(h * HC, (h + 1) * HC)
            nc.scalar.activation(v[:, hs, :], u[:, hs, :], AF.Abs, bias=cbm2[:], scale=1.0)
            nc.scalar.activation(z[:, hs, :], v[:, hs, :], AF.Relu, bias=cb2[:], scale=-1.0)
            nc.scalar.activation(y[:, hs, :], v[:, hs, :], AF.Relu, bias=cb1[:], scale=-1.0)
            if USE_POW:
                nc.vector.tensor_scalar(z3[:, hs, :], z[:, hs, :], 3.0, None, OP.pow)
                nc.vector.tensor_scalar(y3[:, hs, :], y[:, hs, :], 3.0, None, OP.pow)
            else:
                nc.scalar.activation(z2[:, hs, :], z[:, hs, :], AF.Square, bias=cb0[:], scale=1.0)
                nc.scalar.activation(y2[:, hs, :], y[:, hs, :], AF.Square, bias=cb0[:], scale=1.0)
                nc.vector.tensor_mul(z3[:, hs, :], z2[:, hs, :], z[:, hs, :])
                nc.vector.tensor_mul(y3[:, hs, :], y2[:, hs, :], y[:, hs, :])
            # phi = (y3 * -4) + z3
            nc.vector.scalar_tensor_tensor(
                phi[:, hs, :], y3[:, hs, :], -4.0, z3[:, hs, :], OP.mult, OP.add
            )

        # ---- fold mask/scales into weights ----
        mss6 = pool.tile([IN, OUT], F32)
        nc.vector.scalar_tensor_tensor(
            mss6[:], sst[:], 1.0 / 6.0, mk[:], OP.mult, OP.mult
        )
        msb = pool.tile([IN, OUT], MMT)
        nc.vector.scalar_tensor_tensor(msb[:], sbt[:], 0.5, mk[:], OP.mult, OP.mult)
        cwm = pool.tile([IN, GK, OUT], MMT)
        for c in range(GK):
            nc.gpsimd.tensor_mul(cwm[:, c, :], cw[:, c, :], mss6[:])

        # ---- matmuls: silu term + 8 spline terms accumulate in PSUM ----
        ps = psum.tile([OUT, BL], F32)
        nc.tensor.matmul(ps[:], msb[:], sx[:], start=True, stop=False)
        for c in range(GK):
            nc.tensor.matmul(
                ps[:],
                cwm[:, c, :],
                phi[:, c, :],
                start=False,
                stop=(c == GK - 1),
            )

        # ---- + bias, PSUM -> SBUF -> DRAM ----
        ob = pool.tile([OUT, BL], F32)
        nc.scalar.activation(ob[:], ps[:], AF.Identity, bias=bi[:], scale=1.0)
        nc.sync.dma_start(out_d, ob[:])

    nc.compile()
    return nc


def _prep_in_maps(x, coeff, mask, scale_base, scale_spline, bias):
    f32 = np.float32
    x = np.asarray(x, f32)
    coeff = np.asarray(coeff, f32)
    mask = np.asarray(mask, f32)
    scale_base = np.asarray(scale_base, f32)
    scale_spline = np.asarray(scale_spline, f32)
    bias = np.asarray(bias, f32)

    xT = np.ascontiguousarray(x.T)  # [IN, B]
    # coeff[s, c], s = o*IN + i  ->  [i, c, o]
    coefft = np.ascontiguousarray(coeff.reshape(OUT, IN, GK).transpose(1, 2, 0))
    scales3 = np.ascontiguousarray(
        np.stack(
            [
                mask.reshape(OUT, IN).T,
                scale_base.reshape(OUT, IN).T,
                scale_spline.reshape(OUT, IN).T,
            ],
            axis=1,
        )
    )  # [IN, 3, OUT]
    biasp = np.ascontiguousarray(bias.reshape(OUT, 1))

    in_maps = []
    for j in range(NCORE):
        in_maps.append(
            {
                "xt": np.ascontiguousarray(xT[:, j * BL : (j + 1) * BL]),
                "coefft": coefft,
                "scales3": scales3,
                "biasp": biasp,
            }
        )
    return in_maps


def run(x, coeff, mask, scale_base, scale_spline, bias, trace=False):
    if "nc" not in _CACHE:
        _CACHE["nc"] = _build_nc()
    nc = _CACHE["nc"]
    in_maps = _prep_in_maps(x, coeff, mask, scale_base, scale_spline, bias)
    res = run_bass_kernel_spmd(
        nc, in_maps, core_ids=list(range(NCORE)), trace=trace
    )
    outT = np.concatenate(
        [res.results[j]["outt"] for j in range(NCORE)], axis=1
    )  # [OUT, B]
    return np.ascontiguousarray(outT.T), res


def kernel(x, grid, coeff, mask, scale_base, scale_spline, bias, k):
    assert int(np.asarray(k)) == 3
    out, _ = run(x, coeff, mask, scale_base, scale_spline, bias, trace=False)
    return out
